# revision 28
# baseline (speedup 1.0000x reference)
"""NodeNet GNN message-passing kernel for 8 Trainium2 NeuronCores.

Strategy (per sharding hint): shard nodes across the 8 cores; partition
edges by destination node on the host so the scatter-mean is device-local.

v5 - contiguous edge stream + region-coded one-hot (DMA-roofline focus;
the kernel is HBM-bound at ~91us/core):
  - Host balances nodes across cores by edge count (rank-blocks of 8,
    largest count to least-loaded core): all cores carry exactly ~E/8
    edges and near-identical degree profiles.
  - Windows are every W consecutive degree-ranks. Their edge spans come
    from the shared rank-wise MAX profile, so the layout is identical
    across cores (SPMD) while per-core padding is only the tiny profile
    excess plus per-group chunk alignment: ~0.5% edge padding vs ~6%
    for capacity-granular packing.
  - Every segment-sum matmul uses the FULL 128-partition chunk; windows
    sharing a chunk are disambiguated by REGION-coded dstrel values
    (code = W*region + slot, where region = the window's index among
    windows intersecting that chunk; a window continuing from a prior
    chunk is always region 0). The one-hot is built per region level as
    one rectangular 2x is_equal over the level's chunk span against an
    iota slice.
  - Edge rows are pre-scaled by 1/count(dst) and cast to fp8 e4m3, so
    the device segment-sum directly yields the mean. dstrel ships as
    fp16 (region codes exceed fp8's exact-integer range); the idle Pool
    engine expands it to a packed wide tile for the DVE 2x is_equal.
  - attr DMAs are batched 2 processed-groups per instruction and x
    DMAs 4 (consecutive processed groups are reverse-contiguous in
    DRAM): fewer, larger DMAs keep the scheduler's queue model ahead of
    the compute ladder so it interleaves stages of adjacent groups
    instead of serializing ladders.
  - The 3-layer MLP runs feature-major per 512-slot group, software-
    pipelined across 6 issue stages; PSUM evacuations split ScalarE
    (mean, h1a, h2a) / VectorE (h1b, h2b, out); outputs accumulate in
    one SBUF tile and are flushed as 2-group slabs from the Pool SWDGE
    queue mid-stream and from the then-idle SP queue during the drain
    (skips the 994ns SWDGE desc-gen on the final critical chain).
  - Group sizes taper at both processing ends (head 26,50; tail
    48,32,22,14,9,6 in 64ths) so pipeline fill and drain chains stay
    short.

Cost-model timeline (per core): DMA 91.1us busy (32.7MB at 360GB/s,
92.7% occupancy; 0.7% edge padding), DVE 68.0us, PE 61.9us, ACT 49.2us,
Pool 33.2us; wall 98.3us vs the 107.7us session-start baseline.
"""

import numpy as np
import ml_dtypes

import concourse.bacc as bacc
import concourse.mybir as mybir
import concourse.tile as tile
from concourse.bass_utils import run_bass_kernel_spmd

P = 128                    # partitions / matmul contraction tile
D = 128                    # node & edge feature dim
HIDDEN = 256
DOUT = 128
N_NODES = 100000
N_CORES = 8
NPC_REAL = 12500           # real nodes per core
W = 8                      # node slots per window (one-hot code block)
GROUP_SLOTS = 512          # node slots per pipeline group (PSUM tile cols)
SENT = 448.0               # dstrel sentinel (max e4m3; != any code)
# region-slot code table: 32 integers exactly representable in fp8 e4m3
CODE_TABLE = np.array(
    list(range(17)) + list(range(18, 34, 2)) + list(range(36, 61, 4)),
    np.float32)[:32]

_prog_cache: dict = {}
_LABEL = ""     # current build phase, for offline schedule analysis

# engine assignment for the six PSUM evacuations (chain cadence tuning)
EVAC = {"mean": "A", "h1a": "A", "h1b": "V", "h2a": "A", "h2b": "V", "og": "V"}
EXP_ENG = "P"     # dstrel expand: "A" ScalarE, "P" Pool, "V" DVE
STORE_ENG = "P"   # slab stores: "P" Pool SWDGE, "S" SP queue
STORE_SLAB = 2    # flush cadence in completed groups
OH_AHEAD = 2      # one-hot groups built ahead of use
ATTR_PAIR = 2     # processed groups per attr DMA (contiguous span)
X_QUAD = 4        # processed groups per x DMA (contiguous span)
ATTR_BUFS = 4
OH_BUFS = 5
PBIN_BUFS = 2
PMLP_BUFS = 1
PO_BUFS = 2
ACT_BUFS = 8
DW_BUFS = 3
X_BUFS = 3
# processing-order group-size tapers, in 64ths of a full group
HEAD_TAPER = (26, 50)
TAIL_TAPER = (48, 32, 22, 14, 9, 6)

f32 = mybir.dt.float32
f16 = mybir.dt.float16
f8 = mybir.dt.float8e4

Relu = mybir.ActivationFunctionType.Relu
Ident = mybir.ActivationFunctionType.Identity


def _evac(nc, key, out, in_, bias, relu=True):
    """PSUM->SBUF evacuation with bias (+relu) on ACT or DVE per EVAC.
    Mode "S" splits the columns: ACT takes the first half, DVE the
    second, halving that chain hop's latency."""
    if EVAC[key] == "S":
        ncols = out.shape[-1]
        h = (ncols // 2 + 1) & ~1
        saved = dict(EVAC)
        try:
            EVAC[key] = "A"
            _evac(nc, key, out[:, :h], in_[:, :h], bias, relu)
            EVAC[key] = "V"
            _evac(nc, key, out[:, h:], in_[:, h:], bias, relu)
        finally:
            EVAC.update(saved)
        return
    if EVAC[key] == "A":
        if bias is None:
            nc.scalar.copy(out=out, in_=in_)
        else:
            nc.scalar.activation(out=out, in_=in_,
                                 func=(Relu if relu else Ident), bias=bias)
    else:
        if bias is None:
            nc.vector.tensor_copy(out=out, in_=in_)
        elif relu:
            nc.vector.tensor_scalar(
                out=out, in0=in_, scalar1=bias, scalar2=0.0,
                op0=mybir.AluOpType.add, op1=mybir.AluOpType.max,
            )
        else:
            nc.vector.tensor_scalar(
                out=out, in0=in_, scalar1=bias, scalar2=None,
                op0=mybir.AluOpType.add,
            )


def _group_plan(n_windows):
    """Group sizes in windows (plan order). Processing order is the
    reverse; HEAD_TAPER sizes start the pipeline, TAIL_TAPER sizes end
    it so the final drain chains are short."""
    gpw = GROUP_SLOTS // W
    head = [max(1, (h * gpw) // 64) for h in HEAD_TAPER]
    tailr = [max(1, (t * gpw) // 64) for t in TAIL_TAPER]
    gsizes = list(tailr[::-1])
    rem = n_windows - sum(gsizes) - sum(head)
    while rem >= gpw:
        gsizes.append(gpw)
        rem -= gpw
    if rem:
        gsizes.append(rem)
    return gsizes + head[::-1]


def _build_layout(profile):
    """Shared SPMD edge-stream layout from the rank-wise max profile.

    Returns (META, wstart, wsum, rw0):
      META = (gbounds, wparts, gchunks, regops, ohws, NCH, n_windows)
        gbounds: per-group (w0, w1) window ranges (plan order)
        wparts:  per-window [(chunk, oh column offset)] matmul parts
        gchunks: per-group (c0, c1) chunk ranges (disjoint, contiguous)
        regops:  per-group [(r, cb, cspan, ohoff)] one-hot region ops
                 (cb is group-local)
        ohws:    per-group one-hot tile width in columns
      wstart: per-window edge-stream start offset (shared)
      wsum:   per-window edge capacity
      rw0:    per-window region index within its first chunk
    """
    n_windows = (NPC_REAL + W - 1) // W
    alloc = np.zeros(n_windows * W, np.int64)
    alloc[:NPC_REAL] = profile
    wsum = alloc.reshape(n_windows, W).sum(axis=1)

    gsizes = _group_plan(n_windows)
    gbounds = []
    w = 0
    for gs in gsizes:
        gbounds.append((w, w + gs))
        w += gs
    assert w == n_windows, (w, n_windows)

    # group-aligned edge stream: each group's span pads to 128 rows
    wstart = np.zeros(n_windows + 1, np.int64)
    gchunks = []
    base = 0
    for w0, w1 in gbounds:
        c0 = base // 128
        e = base
        for j in range(w0, w1):
            wstart[j] = e
            e += wsum[j]
        base = ((e + 127) // 128) * 128
        gchunks.append((c0, base // 128))
    wstart[n_windows] = base
    NCH = base // 128

    wparts = [[] for _ in range(n_windows)]
    regops = []
    ohws = []
    rw0 = np.zeros(n_windows, np.int64)
    for gi, (w0, w1) in enumerate(gbounds):
        c0, c1 = gchunks[gi]
        nreg = np.zeros(c1 - c0, np.int64)
        parts_rc = {}
        for j in range(w0, w1):   # ascending start order
            if wsum[j] == 0:
                continue
            ca = wstart[j] // 128
            cb = (wstart[j] + wsum[j] - 1) // 128
            for c in range(ca, cb + 1):
                if c == ca and wstart[j] % 128 != 0:
                    r = int(nreg[c - c0])   # mid-chunk starter
                else:
                    r = 0                   # continues into c / starts at 0
                nreg[c - c0] = max(nreg[c - c0], r + 1)
                parts_rc[(j, c)] = r
                if c == ca:
                    rw0[j] = r
        ops = []
        off = 0
        roff = {}
        rmax = int(nreg.max()) if len(nreg) else 0
        for r in range(rmax):
            has = np.nonzero(nreg > r)[0]
            cb_ = int(has.min())
            ce_ = int(has.max()) + 1
            ops.append((r, cb_, ce_ - cb_, off))
            roff[r] = (cb_, off)
            off += (ce_ - cb_) * W
        regops.append(tuple(ops))
        ohws.append(off)
        for (j, c), r in parts_rc.items():
            cb_, off_r = roff[r]
            ohoff = off_r + (c - c0 - cb_) * W
            wparts[j].append((c, ohoff))
        for j in range(w0, w1):
            wparts[j].sort()
    META = (
        tuple(gbounds),
        tuple(tuple(p) for p in wparts),
        tuple(gchunks),
        tuple(regops),
        tuple(ohws),
        NCH,
        n_windows,
    )
    return META, wstart, wsum, rw0


def _build_program(META):
    gbounds, wparts, gchunks, regops, ohws, NCH, n_windows = META
    NPC = n_windows * W
    CBG_max = max(c1 - c0 for c0, c1 in gchunks)
    NW_max = max(w1 - w0 for w0, w1 in gbounds) * W
    OHW_max = max(ohws)
    RW_max = max(op[0] + 1 for ops in regops for op in ops) * W

    nc = bacc.Bacc(None)
    attr8_d = nc.dram_tensor("attr8", [P, NCH * D], f8, kind="ExternalInput")
    x16_d = nc.dram_tensor("x16", [P, NPC], f16, kind="ExternalInput")
    d8_d = nc.dram_tensor("d8", [P, NCH], f8, kind="ExternalInput")
    c16_d = nc.dram_tensor("c16", [P, RW_max], f16, kind="ExternalInput")
    consts_d = nc.dram_tensor("consts", [P, 5], f32, kind="ExternalInput")
    wts_d = nc.dram_tensor("wts", [P, 4 * HIDDEN + 2 * DOUT], f16,
                           kind="ExternalInput")
    outT_d = nc.dram_tensor("outT", [P, NPC], f16, kind="ExternalOutput")

    # processing order: reversed plan order; consecutive processed groups
    # are reverse-contiguous in both the chunk and the slot streams
    porder = list(range(len(gbounds)))[::-1]

    with tile.TileContext(nc) as tc:
        with (
            tc.tile_pool(name="const", bufs=1) as cpool,
            tc.tile_pool(name="attr", bufs=ATTR_BUFS) as apool,
            tc.tile_pool(name="x", bufs=X_BUFS) as xpool,
            tc.tile_pool(name="oh", bufs=OH_BUFS) as ohpool,
            tc.tile_pool(name="dw", bufs=DW_BUFS) as dwpool,
            tc.tile_pool(name="acts", bufs=ACT_BUFS) as actpool,
            tc.tile_pool(name="pbin", bufs=PBIN_BUFS, space="PSUM") as pbin,
            tc.tile_pool(name="pmlp", bufs=PMLP_BUFS, space="PSUM") as pmlp,
            tc.tile_pool(name="ppo", bufs=PO_BUFS, space="PSUM") as ppo,
        ):
            cs = cpool.tile([P, 5], f32, tag="consts")
            ws = cpool.tile([P, 4 * HIDDEN + 2 * DOUT], f16, tag="wts")
            it16 = cpool.tile([P, RW_max], f16, tag="c16")
            d8s = cpool.tile([P, NCH], f8, tag="d8")
            w1s_0 = ws[:, 0:HIDDEN]
            w1s_1 = ws[:, HIDDEN : 2 * HIDDEN]
            w2s_0 = ws[:, 2 * HIDDEN : 3 * HIDDEN]
            w2s_1 = ws[:, 3 * HIDDEN : 4 * HIDDEN]
            w3s_0 = ws[:, 4 * HIDDEN : 4 * HIDDEN + DOUT]
            w3s_1 = ws[:, 4 * HIDDEN + DOUT : 4 * HIDDEN + 2 * DOUT]
            b1s_0 = cs[:, 0:1]
            b1s_1 = cs[:, 1:2]
            b2s_0 = cs[:, 2:3]
            b2s_1 = cs[:, 3:4]
            b3s = cs[:, 4:5]
            oall = cpool.tile([P, NPC], f16, tag="oall")

            store_dma = {"P": nc.gpsimd.dma_start,
                         "S": nc.sync.dma_start,
                         "A": nc.scalar.dma_start,
                         "V": nc.vector.dma_start}[STORE_ENG]

            def build_oh(g):
                """One-hot for plan-group g: expand the fp16 dstrel codes
                to a packed wide tile (a stride-0 last-dim broadcast
                would deny DVE its 2x mode), then one 2x is_equal per
                region level against the matching iota slice (broadcast
                middle dim keeps the last dim packed)."""
                global _LABEL
                _LABEL = f"oh:{g}"
                c0, c1 = gchunks[g]
                CBg = c1 - c0
                oh = ohpool.tile([P, OHW_max], f16, tag="oh")
                dw = dwpool.tile([P, CBG_max * W], f16, tag="dw")
                exp_copy = {"A": nc.scalar.copy, "P": nc.gpsimd.tensor_copy,
                            "V": nc.vector.tensor_copy}[EXP_ENG]
                exp_copy(
                    out=dw[:, : CBg * W].rearrange("p (c m) -> p c m", m=W),
                    in_=d8s[:, c0:c1].to_broadcast([P, CBg, W]),
                )
                for r, cb, cspan, ohoff in regops[g]:
                    iota_b = (it16[:, r * W : (r + 1) * W]
                              .unsqueeze(1).to_broadcast([P, cspan, W]))
                    nc.vector.tensor_tensor(
                        out=oh[:, ohoff : ohoff + cspan * W]
                            .rearrange("p (c m) -> p c m", m=W),
                        in0=dw[:, cb * W : (cb + cspan) * W]
                            .rearrange("p (c m) -> p c m", m=W),
                        in1=iota_b,
                        op=mybir.AluOpType.is_equal,
                    )
                return oh

            glist = list(porder)
            ngl = len(glist)
            st = [dict() for _ in range(ngl)]
            oh_tiles = {}
            built = -1

            def ensure_oh(through_k):
                nonlocal built
                while built < min(through_k, ngl - 1):
                    built += 1
                    oh_tiles[built] = build_oh(glist[built])

            # --- software-pipelined issue: stages one group apart so no
            # in-order engine queue stalls on a same-group ladder ---

            def s0_feed(k):
                global _LABEL
                _LABEL = f"s0:{k}"
                g = glist[k]
                w0, w1 = gbounds[g]
                NW = (w1 - w0) * W
                s = st[k]
                s["NW"] = NW
                s["n0"] = w0 * W
                if k % ATTR_PAIR == 0:
                    k1 = min(k + ATTR_PAIR, ngl) - 1
                    blo = gchunks[glist[k1]][0]
                    bhi = gchunks[g][1]
                    at8 = apool.tile([P, ATTR_PAIR * CBG_max * D], f8,
                                     tag="attr")
                    nc.sync.dma_start(
                        out=at8[:, : (bhi - blo) * D],
                        in_=attr8_d[:, blo * D : bhi * D],
                    )
                    for j in range(k, k1 + 1):
                        st[j]["at8"] = at8
                        st[j]["cbase"] = blo
                if k == 0:
                    nc.sync.dma_start(out=d8s[:], in_=d8_d[:, :])
                    nc.sync.dma_start(out=it16[:], in_=c16_d[:, :])
                    nc.sync.dma_start(out=cs[:], in_=consts_d[:, :])
                    nc.sync.dma_start(out=ws[:], in_=wts_d[:, :])
                if k % X_QUAD == 0:
                    k1 = min(k + X_QUAD, ngl) - 1
                    nlo = gbounds[glist[k1]][0] * W
                    nhi = s["n0"] + NW
                    xg = xpool.tile([P, X_QUAD * NW_max], f16, tag="x")
                    nc.sync.dma_start(out=xg[:, : nhi - nlo],
                                      in_=x16_d[:, nlo:nhi])
                    for j in range(k, k1 + 1):
                        st[j]["xg"] = xg
                        st[j]["nbase"] = nlo
                ensure_oh(k + OH_AHEAD)

            def s1_seg(k):
                global _LABEL
                _LABEL = f"s1:{k}"
                g = glist[k]
                s = st[k]
                NW = s["NW"]
                w0, w1 = gbounds[g]
                oh = oh_tiles[k]
                at8 = s["at8"]
                cbase = s["cbase"]
                pm = pbin.tile([P, NW_max], f32, tag="pm")
                for w in range(w0, w1):
                    parts = wparts[w]
                    sw = w - w0
                    for i, (c, ohoff) in enumerate(parts):
                        lc = c - cbase
                        nc.tensor.matmul(
                            out=pm[:, sw * W : (sw + 1) * W],
                            lhsT=at8[:, lc * D : (lc + 1) * D],
                            rhs=oh[:, ohoff : ohoff + W],
                            start=(i == 0),
                            stop=(i == len(parts) - 1),
                        )
                mean_g = actpool.tile([P, NW_max], f16, tag="mean_g")
                _evac(nc, "mean", mean_g[:, :NW], pm[:, :NW], None)
                s["mean"] = mean_g
                s["at8"] = None

            def s2_l1(k):
                global _LABEL
                _LABEL = f"s2:{k}"
                s = st[k]
                NW = s["NW"]
                o = s["n0"] - s["nbase"]
                xg = s["xg"][:, o : o + NW]
                mean_g = s["mean"]
                ph1a = pmlp.tile([P, NW_max], f32, tag="h1a")
                ph1b = pmlp.tile([P, NW_max], f32, tag="h1b")
                nc.tensor.matmul(out=ph1a[:, :NW], lhsT=w1s_0[:, 0:P],
                                 rhs=xg, start=True, stop=False)
                nc.tensor.matmul(out=ph1a[:, :NW], lhsT=w1s_1[:, 0:P],
                                 rhs=mean_g[:, :NW], start=False, stop=True)
                nc.tensor.matmul(out=ph1b[:, :NW], lhsT=w1s_0[:, P:HIDDEN],
                                 rhs=xg, start=True, stop=False)
                nc.tensor.matmul(out=ph1b[:, :NW], lhsT=w1s_1[:, P:HIDDEN],
                                 rhs=mean_g[:, :NW], start=False, stop=True)
                h1a = actpool.tile([P, NW_max], f16, tag="h1a_s")
                h1b = actpool.tile([P, NW_max], f16, tag="h1b_s")
                _evac(nc, "h1a", h1a[:, :NW], ph1a[:, :NW], b1s_0[:, 0:1])
                _evac(nc, "h1b", h1b[:, :NW], ph1b[:, :NW], b1s_1[:, 0:1])
                s["h1a"], s["h1b"] = h1a, h1b
                s["xg"] = s["mean"] = None

            def s3_l2(k):
                global _LABEL
                _LABEL = f"s3:{k}"
                s = st[k]
                NW = s["NW"]
                h1a, h1b = s["h1a"], s["h1b"]
                ph2a = pmlp.tile([P, NW_max], f32, tag="h2a")
                ph2b = pmlp.tile([P, NW_max], f32, tag="h2b")
                nc.tensor.matmul(out=ph2a[:, :NW], lhsT=w2s_0[:, 0:P],
                                 rhs=h1a[:, :NW], start=True, stop=False)
                nc.tensor.matmul(out=ph2a[:, :NW], lhsT=w2s_1[:, 0:P],
                                 rhs=h1b[:, :NW], start=False, stop=True)
                nc.tensor.matmul(out=ph2b[:, :NW], lhsT=w2s_0[:, P:HIDDEN],
                                 rhs=h1a[:, :NW], start=True, stop=False)
                nc.tensor.matmul(out=ph2b[:, :NW], lhsT=w2s_1[:, P:HIDDEN],
                                 rhs=h1b[:, :NW], start=False, stop=True)
                h2a = actpool.tile([P, NW_max], f16, tag="h2a_s")
                h2b = actpool.tile([P, NW_max], f16, tag="h2b_s")
                _evac(nc, "h2a", h2a[:, :NW], ph2a[:, :NW], b2s_0[:, 0:1])
                _evac(nc, "h2b", h2b[:, :NW], ph2b[:, :NW], b2s_1[:, 0:1])
                s["h2a"], s["h2b"] = h2a, h2b
                s["h1a"] = s["h1b"] = None

            def s4_l3(k):
                global _LABEL
                _LABEL = f"s4:{k}"
                s = st[k]
                NW = s["NW"]
                n0 = s["n0"]
                h2a, h2b = s["h2a"], s["h2b"]
                po = ppo.tile([P, NW_max], f32, tag="po")
                nc.tensor.matmul(out=po[:, :NW], lhsT=w3s_0[:],
                                 rhs=h2a[:, :NW], start=True, stop=False)
                nc.tensor.matmul(out=po[:, :NW], lhsT=w3s_1[:],
                                 rhs=h2b[:, :NW], start=False, stop=True)
                _evac(nc, "og", oall[:, n0 : n0 + NW], po[:, :NW],
                      b3s[:, 0:1], relu=False)
                s["h2a"] = s["h2b"] = None

            def s5_store(k):
                global _LABEL
                _LABEL = f"s5:{k}"
                if (k + 1) % STORE_SLAB and k != ngl - 1:
                    return
                k0 = (k // STORE_SLAB) * STORE_SLAB
                lo = min(st[j]["n0"] for j in range(k0, k + 1))
                hi = max(st[j]["n0"] + st[j]["NW"] for j in range(k0, k + 1))
                # drain-phase stores go via the idle SP HWDGE queue (no
                # attr DMAs left to block; skips the 994ns SWDGE gen)
                dma = nc.sync.dma_start if k >= ngl - 5 else store_dma
                dma(out=outT_d[:, lo:hi], in_=oall[:, lo:hi])

            stages = (s0_feed, s1_seg, s2_l1, s3_l2, s4_l3, s5_store)
            nstg = len(stages)
            for it in range(ngl + nstg - 1):
                for si in range(nstg):
                    k = it - si
                    if 0 <= k < ngl:
                        stages[si](k)

    nc.finalize()
    return nc


def _host_prep(x, edge_index, edge_attr):
    """Sort/scale/pack edges into the shared contiguous layout; returns
    (META, per-core input arrays, per-core slot->global-node maps)."""
    col = np.asarray(edge_index)[1].astype(np.int64)
    x = np.asarray(x, dtype=np.float32)
    counts = np.bincount(col, minlength=N_NODES).astype(np.int64)
    scale = (1.0 / np.maximum(counts, 1)).astype(np.float32)

    eorder = np.argsort(col, kind="stable")
    col_s = col[eorder]
    attr_s = np.asarray(edge_attr, dtype=np.float32)[eorder]
    attr_s = attr_s * scale[col_s][:, None]
    attr_s8 = attr_s.astype(ml_dtypes.float8_e4m3)
    estart = np.zeros(N_NODES + 1, np.int64)
    estart[1:] = np.cumsum(counts)

    # balanced node->core assignment: rank blocks of N_CORES, biggest
    # count in the block to the least-loaded core; per-core profiles are
    # then nearly identical and edge totals balance to ~E/8
    gorder = np.argsort(-counts, kind="stable")
    nodetbl = np.empty((N_CORES, NPC_REAL), np.int64)
    loads = np.zeros(N_CORES, np.int64)
    for b in range(NPC_REAL):
        blk = gorder[b * N_CORES : (b + 1) * N_CORES]
        asc = np.argsort(loads, kind="stable")
        nodetbl[asc, b] = blk
        loads[asc] += counts[blk]
    cnt_sorted = counts[nodetbl]
    profile = cnt_sorted.max(axis=0)

    META, wstart, wsum, rw0 = _build_layout(profile)
    gbounds, wparts, gchunks, regops, ohws, NCH, n_windows = META
    E_pad = NCH * 128
    NPC = n_windows * W

    nranks = n_windows * W
    ranks = np.arange(nranks)
    win_of_rank = ranks // W
    slot_of_rank = ranks % W
    wfirst = wstart[:n_windows] // 128
    midstart = (wstart[:n_windows] % 128) != 0

    per_core = []
    slot_node = []
    for c in range(N_CORES):
        cnts = np.zeros(nranks, np.int64)
        cnts[:NPC_REAL] = cnt_sorted[c]
        # own edges pack contiguously from the (shared) window start
        inwin = cnts.reshape(n_windows, W)
        prew = np.cumsum(inwin, axis=1) - inwin
        ebase = (wstart[:n_windows, None] + prew).reshape(-1)

        total = int(cnts.sum())
        rk = np.repeat(ranks, cnts)
        within = np.arange(total) - np.repeat(np.cumsum(cnts) - cnts, cnts)
        pos = np.repeat(ebase, cnts) + within
        gnode_r = np.full(nranks, -1, np.int64)
        gnode_r[:NPC_REAL] = nodetbl[c]
        src_idx = estart[gnode_r[rk]] + within

        attr_pad = np.zeros((E_pad, D), ml_dtypes.float8_e4m3)
        attr_pad[pos] = attr_s8[src_idx]
        attr8 = np.ascontiguousarray(
            attr_pad.reshape(NCH, P, D).transpose(1, 0, 2).reshape(P, NCH * D)
        )

        chunk_of = pos // 128
        w_of = win_of_rank[rk]
        r_of = np.where(
            (chunk_of == wfirst[w_of]) & midstart[w_of], rw0[w_of], 0
        )
        code = CODE_TABLE[W * r_of + slot_of_rank[rk]]

        dstrel = np.full((E_pad,), SENT, ml_dtypes.float8_e4m3)
        dstrel[pos] = code.astype(ml_dtypes.float8_e4m3)
        dstrelT = np.ascontiguousarray(dstrel.reshape(NCH, P).T)

        smap = np.full(NPC, -1, np.int64)
        smap[:NPC_REAL] = nodetbl[c]
        xT = np.zeros((NPC, D), np.float16)
        xT[:NPC_REAL] = x[nodetbl[c]].astype(np.float16)
        xT = np.ascontiguousarray(xT.T)

        per_core.append({"attr8": attr8, "dstrelT": dstrelT, "x16": xT})
        slot_node.append(smap)
    return META, per_core, slot_node


def _build_consts(b1, b2, b3):
    consts = np.zeros((P, 5), np.float32)
    consts[:, 0] = b1[:P]
    consts[:, 1] = b1[P:]
    consts[:, 2] = b2[:P]
    consts[:, 3] = b2[P:]
    consts[:, 4] = b3
    return consts


def _build_wts(W1, W2, W3):
    wts = np.empty((P, 4 * HIDDEN + 2 * DOUT), np.float16)
    wts[:, 0:HIDDEN] = W1[:P]
    wts[:, HIDDEN : 2 * HIDDEN] = W1[P:]
    wts[:, 2 * HIDDEN : 3 * HIDDEN] = W2[:P]
    wts[:, 3 * HIDDEN : 4 * HIDDEN] = W2[P:]
    wts[:, 4 * HIDDEN : 4 * HIDDEN + DOUT] = W3[:P]
    wts[:, 4 * HIDDEN + DOUT : 4 * HIDDEN + 2 * DOUT] = W3[P:]
    return wts


def _build_c16(META):
    """fp16 code ramp [P, RW_max] (remapped region-slot code values)."""
    regops = META[3]
    RW_max = max(op[0] + 1 for ops in regops for op in ops) * W
    return np.tile(CODE_TABLE[:RW_max].astype(np.float16), (P, 1))


def _make_in_maps(META, per_core, b1, b2, b3, W1, W2, W3):
    consts = _build_consts(b1, b2, b3)
    wts = _build_wts(W1, W2, W3)
    c16 = _build_c16(META)
    return [
        {
            "attr8": pc["attr8"].view(np.uint8),
            "x16": pc["x16"],
            "d8": pc["dstrelT"].view(np.uint8),
            "c16": c16,
            "consts": consts,
            "wts": wts,
        }
        for pc in per_core
    ]


def kernel(x, edge_index, edge_attr, W1, b1, W2, b2, W3, b3):
    META, per_core, slot_node = _host_prep(x, edge_index, edge_attr)

    if META not in _prog_cache:
        _prog_cache[META] = _build_program(META)
    nc = _prog_cache[META]

    in_maps = _make_in_maps(
        META, per_core,
        np.asarray(b1, np.float32), np.asarray(b2, np.float32),
        np.asarray(b3, np.float32),
        np.asarray(W1, np.float32), np.asarray(W2, np.float32),
        np.asarray(W3, np.float32),
    )
    res = run_bass_kernel_spmd(nc, in_maps, core_ids=list(range(N_CORES)))

    out = np.empty((N_NODES, DOUT), np.float32)
    for c in range(N_CORES):
        o = res.results[c]["outT"].T.astype(np.float32)
        smap = slot_node[c]
        m = smap >= 0
        out[smap[m]] = o[m]
    return out
